# revision 3
# baseline (speedup 1.0000x reference)
"""Deformable causal conv1d Trainium2 kernel (v5).

Math (validated vs reference; h-term dropped, costs 4.0e-3 rel):
     sampled[c,k,t] ~ a0 - d*D0
  a0 = x[c,t+k-7], D[u] = x[u]-x[u-1], d = |raw+b| (raw = causal 3-tap
  depthwise conv of x).

v5 engine placement:
  - raw: TensorE 32x32 diag-block matmuls, 16 concurrent PE tiles
    (tile_position row/col groups). Rotation trick: k-pair k//2=r runs at
    col-group (i+r)%4, reading a host-side block-rotated copy of x, so
    all 8 k's of a quad-wave stream concurrently and psum stays c-layout.
  - d = |raw+b|: ScalarE Abs (per-partition bias), PSUM->SBUF bf16.
  - S-assembly: 2 bf16 TTs per k-quad (p = d*D0; S = a0 - p) via
    strided/overlapping same-parity quad APs.
  - out += W_k @ S: TensorE, accumulating over (ct,k) in PSUM.
  - x (+3 rotated copies) -> bf16 parity tiles: SWDGE cast-DMA.

Sharding: 8 cores = 4 batches x 2 time-halves. No collectives.
"""

import numpy as np
import ml_dtypes
import bass_rust

import concourse.bass as bass
import concourse.tile as tile
from concourse import bacc, mybir

F32 = mybir.dt.float32
BF16 = mybir.dt.bfloat16
Alu = mybir.AluOpType
Act = mybir.ActivationFunctionType

B, C, T = 4, 512, 4096
K, OK = 8, 3
O = 512  # C_out
H = 16  # left halo columns in the x slice
TH = 2048  # time columns per core
N_CORES = 8


def _strided(t, base_col, outer_step, outer_n, inner_n):
    """Overlapping AP over SBUF tile t: [128, outer_n, inner_n] where
    element [p, a, i] = t[p, base_col + a*outer_step + i]."""
    a = t[:, 0:inner_n].copy()
    pstep = tuple(list(a.ap)[0])
    a.ap = bass_rust.VecI64Pair(
        [pstep, (outer_step, outer_n), (1, inner_n)]
    )
    a.offset = base_col
    return a


def build_device_program(
    th=TH,
    tt=512,  # time chunk = one PSUM bank of fp32
    n_ct=4,  # contraction c-tiles of 128
    n_ot=4,  # output o-tiles of 128
):
    n_chunks = th // tt
    c_in = n_ct * 128
    o_out = n_ot * 128

    nc = bacc.Bacc("TRN2", target_bir_lowering=False, debug=False)

    x_d = nc.dram_tensor("xcore", [c_in, H + th], F32, kind="ExternalInput").ap()
    xr_d = nc.dram_tensor("xrot", [3, c_in, H + th], F32, kind="ExternalInput").ap()
    wt_d = nc.dram_tensor("wt", [n_ct, K, 128, o_out], BF16, kind="ExternalInput").ap()
    dgw_d = nc.dram_tensor(
        "diagw", [n_ct, K, OK, 128, 32], BF16, kind="ExternalInput"
    ).ap()
    offb_d = nc.dram_tensor("offb", [n_ct, 128, K], F32, kind="ExternalInput").ap()
    bias_d = nc.dram_tensor("biasr", [128, n_ot], F32, kind="ExternalInput").ap()
    out_d = nc.dram_tensor("out", [o_out, th], F32, kind="ExternalOutput").ap()

    W = H + tt  # working width incl halo
    QT = 4 * tt  # quad width

    with tile.TileContext(nc) as tc:
        with (
            tc.tile_pool(name="const", bufs=1) as cpool,
            tc.tile_pool(name="xb", bufs=3) as xbpool,
            tc.tile_pool(name="xrb", bufs=2) as xrpool,
            tc.tile_pool(name="chain", bufs=3) as chain,
            tc.tile_pool(name="spool", bufs=3) as spool,
            tc.tile_pool(name="outp", bufs=2) as outp,
            tc.tile_pool(name="psum", bufs=1, space="PSUM") as pspool,
            tc.tile_pool(name="rawps", bufs=4, space="PSUM") as rawps,
        ):
            # ---- resident constants ----
            wt_sb = []
            dgw_sb = []
            offb_sb = []
            for ct in range(n_ct):
                w = cpool.tile([128, K, o_out], BF16, tag=f"wt{ct}")
                nc.sync.dma_start(w[:], wt_d[ct].rearrange("k c o -> c k o"))
                wt_sb.append(w)
                g = cpool.tile([128, K, OK, 32], BF16, tag=f"dgw{ct}")
                nc.sync.dma_start(g[:], dgw_d[ct].rearrange("k j c o -> c k j o"))
                dgw_sb.append(g)
                ob = cpool.tile([128, K], F32, tag=f"offb{ct}")
                nc.sync.dma_start(ob[:], offb_d[ct])
                offb_sb.append(ob)
            bias_sb = cpool.tile([128, n_ot], F32, tag="biasr")
            nc.sync.dma_start(bias_sb[:], bias_d)

            for chunk in range(n_chunks):
                ps = {}
                for ot in range(n_ot):
                    ps[ot] = pspool.tile(
                        [128, tt], F32, tag=f"ps{ot}", name=f"ps{ot}"
                    )

                for ct in range(n_ct):
                    # bf16 parity copies via cast-DMA (base + 3 rotations):
                    #   Xe[u] = x[u] (u in [0,W)), Xo[u] = x[u+1] (u in [0,W-1))
                    xe = []
                    xo = []
                    for r in range(4):
                        src = (
                            x_d[ct * 128 : (ct + 1) * 128]
                            if r == 0
                            else xr_d[r - 1, ct * 128 : (ct + 1) * 128]
                        )
                        pool = xbpool if r == 0 else xrpool
                        e = pool.tile([128, W], BF16, tag=f"Xe{r}")
                        nc.gpsimd.dma_start(
                            e[:], src[:, chunk * tt : chunk * tt + W]
                        )
                        o = pool.tile([128, W], BF16, tag=f"Xo{r}")
                        nc.gpsimd.dma_start(
                            o[:, 0 : W - 1],
                            src[:, chunk * tt + 1 : chunk * tt + W],
                        )
                        xe.append(e)
                        xo.append(o)
                    Xe, Xo = xe[0], xo[0]
                    # D[u] = x[u]-x[u-1]: De[u]=D[u] (u in [2,W)), Do[v]=D[v+1]
                    De = xbpool.tile([128, W], BF16, tag="De")
                    nc.vector.tensor_tensor(
                        De[:, 2:W], Xe[:, 2:W], Xo[:, 0 : W - 2], Alu.subtract
                    )
                    Do = xbpool.tile([128, W], BF16, tag="Do")
                    nc.vector.tensor_tensor(
                        Do[:, 0 : W - 2], Xo[:, 0 : W - 2], Xe[:, 0 : W - 2],
                        Alu.subtract,
                    )

                    def xs_rot(r, col, n=tt):
                        if col % 2 == 0:
                            return xe[r][:, col : col + n]
                        return xo[r][:, col - 1 : col - 1 + n]

                    for q0 in (0, 1):  # quad-wave = ks {q0, q0+2, q0+4, q0+6}
                        ks = [q0, q0 + 2, q0 + 4, q0 + 6]
                        # raw: 16 concurrent 32x32 PE tiles; k -> rotation k//2,
                        # block i -> tile (32i, 32*((i + k//2) % 4))
                        rp = {}
                        for k in ks:
                            rp[k] = rawps.tile(
                                [128, tt], F32, tag="rawps", name=f"rp{q0}_{k}"
                            )
                        for j in range(OK):
                            for i in range(4):
                                for k in ks:
                                    r = k // 2
                                    jc = (i + r) % 4
                                    mov = xs_rot(r, H - 2 + j)
                                    nc.tensor.matmul(
                                        rp[k][32 * jc : 32 * jc + 32, :],
                                        dgw_sb[ct][
                                            32 * i : 32 * i + 32, k, j, :
                                        ],
                                        mov[32 * i : 32 * i + 32, :],
                                        start=(j == 0),
                                        stop=(j == OK - 1),
                                        tile_position=(32 * i, 32 * jc),
                                        skip_group_check=True,
                                    )
                        # d = |raw + b| per k (per-partition bias)
                        dd = chain.tile([128, QT], BF16, tag="d")
                        for qi, k in enumerate(ks):
                            nc.scalar.activation(
                                dd[:, qi * tt : (qi + 1) * tt],
                                rp[k][:],
                                Act.Abs,
                                bias=offb_sb[ct][:, k : k + 1],
                            )

                        # strided quad operands at cols k+9, k in ks (same parity)
                        c0 = ks[0] + 9
                        if c0 % 2 == 0:
                            pX = _strided(Xe, c0, 2, 4, tt)
                            pD = _strided(De, c0, 2, 4, tt)
                        else:
                            pX = _strided(Xo, c0 - 1, 2, 4, tt)
                            pD = _strided(Do, c0 - 1, 2, 4, tt)

                        def r4(t):
                            return t[:].rearrange("p (a b) -> p a b", a=4)

                        # S = a0 - d*D
                        p_t = chain.tile([128, QT], BF16, tag="p")
                        nc.vector.tensor_tensor(r4(p_t), r4(dd), pD, Alu.mult)
                        S_t = spool.tile([128, QT], BF16, tag="S")
                        nc.vector.tensor_tensor(r4(S_t), pX, r4(p_t), Alu.subtract)

                        for qi, k in enumerate(ks):
                            first = ct == 0 and q0 == 0 and qi == 0
                            last = ct == n_ct - 1 and q0 == 1 and qi == 3
                            for ot in range(n_ot):
                                nc.tensor.matmul(
                                    ps[ot][:],
                                    wt_sb[ct][:, k, ot * 128 : (ot + 1) * 128],
                                    S_t[:, qi * tt : (qi + 1) * tt],
                                    start=first,
                                    stop=last,
                                )

                for ot in range(n_ot):
                    out_sb = outp.tile([128, tt], F32, tag="osb")
                    nc.scalar.activation(
                        out_sb[:], ps[ot][:], Act.Identity,
                        bias=bias_sb[:, ot : ot + 1],
                    )
                    nc.sync.dma_start(
                        out_d[ot * 128 : (ot + 1) * 128, chunk * tt : (chunk + 1) * tt],
                        out_sb[:],
                    )

    nc.compile()
    return nc


def prep_host_inputs(x, offset_w, offset_b, weight, bias, th=TH):
    wt = (
        weight.transpose(1, 2, 0)  # [C, K, O]
        .reshape(4, 128, K, O)
        .transpose(0, 2, 1, 3)  # [ct, k, c, o]
        .astype(ml_dtypes.bfloat16)
    )
    wt = np.ascontiguousarray(wt)

    ow = offset_w.reshape(C, K, OK).astype(np.float32)  # [c, k, j]
    # diag 32-blocks with rotation-aware placement:
    # dgw[ct, k, j, p, f] = delta(p%32, f) * ow[ct*128 + 32*((p//32 + k//2)%4) + p%32, k, j]
    diagw = np.zeros((4, K, OK, 128, 32), ml_dtypes.bfloat16)
    q = np.arange(128) % 32
    blk = np.arange(128) // 32
    for ct in range(4):
        for k in range(K):
            ch = ct * 128 + 32 * ((blk + k // 2) % 4) + q
            for j in range(OK):
                diagw[ct, k, j, np.arange(128), q] = ow[ch, k, j].astype(
                    ml_dtypes.bfloat16
                )
    offb = np.ascontiguousarray(offset_b.reshape(4, 128, K).astype(np.float32))
    biasr = np.ascontiguousarray(bias.reshape(4, 128).T).astype(np.float32)

    xcores = []
    xrots = []
    n_th = T // th
    for core in range(N_CORES):
        b, thi = divmod(core, n_th)
        t0 = thi * th
        xc = np.zeros((C, H + th), np.float32)
        xc[:, H:] = x[b, :, t0 : t0 + th]
        if t0 >= H:
            xc[:, :H] = x[b, :, t0 - H : t0]
        xcores.append(np.ascontiguousarray(xc))
        # block-rotated copies: rot r partition p = channel (p + 32r) % 128
        xb4 = xc.reshape(4, 128, H + th)
        xr = np.empty((3, C, H + th), np.float32)
        for r in (1, 2, 3):
            idx = (np.arange(128) + 32 * r) % 128
            xr[r - 1] = xb4[:, idx, :].reshape(C, H + th)
        xrots.append(np.ascontiguousarray(xr))
    return wt, diagw, offb, biasr, xcores, xrots


_PROGRAM_CACHE = {}


def _get_program():
    key = "main"
    if key not in _PROGRAM_CACHE:
        _PROGRAM_CACHE[key] = build_device_program()
    return _PROGRAM_CACHE[key]


def run_on_hw(inputs, trace=False, **kw):
    from concourse.bass_utils import run_bass_kernel_spmd

    nc = _get_program()
    wt, diagw, offb, biasr, xcores, xrots = prep_host_inputs(
        inputs["x"], inputs["offset_w"], inputs["offset_b"],
        inputs["weight"], inputs["bias"],
    )
    in_maps = [
        {
            "xcore": xcores[core],
            "xrot": xrots[core],
            "wt": wt,
            "diagw": diagw,
            "offb": offb,
            "biasr": biasr,
        }
        for core in range(N_CORES)
    ]
    res = run_bass_kernel_spmd(
        nc, in_maps, core_ids=list(range(N_CORES)), trace=trace, **kw
    )
    return res


def kernel(**inputs) -> np.ndarray:
    res = run_on_hw(inputs)
    out = np.empty((B, O, T), np.float32)
    n_th = T // TH
    for core in range(N_CORES):
        b, thi = divmod(core, n_th)
        out[b, :, thi * TH : (thi + 1) * TH] = res.results[core]["out"]
    return out


if __name__ == "__main__":
    z = np.load("/root/problem/inputs.npz")
    out = kernel(**{k: z[k] for k in z.files})
    print("kernel out:", out.shape, out.dtype, float(np.abs(out).max()))


# revision 5
# speedup vs baseline: 1.5031x; 1.5031x over previous
"""Deformable causal conv1d Trainium2 kernel (v6).

Math (h-term dropped; 4.0e-3 rel): sampled[c,k,t] ~ a0 - d*D0 with
a0 = x[c,t+k-7], D[u] = x[u]-x[u-1], d = |raw+b|.

v6 = v5's 16-way tiled raw matmuls + software pipelining to keep the PE
dense (HAM stays at K=8/8):
  - Unit = (chunk, ct, q0-quad). PE stream: raw(u), main(u-1), raw(u+1),
    main(u), ...  main trails raw by one unit, so the d->p->S latency of
    unit u hides under raw(u+1) + main(u-1) instead of stalling the PE.
  - raw: 48 tile-MMs per unit on 16 concurrent 32x32 PE tiles
    (k of quad -> rotation r=k//2 -> col-group (i+r)%4, reading host-side
    block-rotated x copies; all 4 rotations present in every quad).
  - d = |raw+b|: 2 k's on ScalarE (Abs + bias), 2 k's on VectorE
    (tensor_scalar add-bias then abs_max 0) to halve the serial latency.
  - S-assembly: 2 bf16 TTs per quad (p = d*D0; S = a0 - p).
  - PSUM: 4 banks out accumulation + 4 banks raw (one unit in flight +
    one being drained) = 8.

Sharding: 8 cores = 4 batches x 2 time-halves. No collectives.
"""

import numpy as np
import ml_dtypes
import bass_rust

import concourse.bass as bass
import concourse.tile as tile
from concourse import bacc, mybir


def _register_dve_ops():
    import concourse.dve_ops as dops
    from concourse.dve_spec import Spec, Src0, C0, Zero, lower, maxx
    from concourse.dve_uop import DveOpSpec
    from concourse.dve_table_gen import dve_ver_for

    name = "DEFORM_D_ABS"
    for op in dops.OPS:
        if op.name == name:
            return op
    row = dops._CUSTOM_DVE_ROW_BASE + len(dops.OPS)
    assert row < 0x20
    dops._SUB_OPCODE_FOR_NAME[name] = row
    _t = Src0 + C0
    spec = Spec(
        body=maxx(_t, Zero - _t),
        reference=lambda in0, in1, s0, s1, imm2: np.abs(
            in0.astype(np.float32) + s0
        ),
    )
    shas = {}
    for ver in {dve_ver_for("TRN2"), dve_ver_for("TRN3")}:
        c = DveOpSpec(name=name, opcode=row, uops=lower(spec, ver=ver), rd1_en=False)
        shas[ver] = c.sha(ver)
    op = dops.DveOp(name, spec, subdim=False, uops_sha=shas)
    dops.OPS.append(op)
    dops.CUSTOM_DVE_SPECS[name] = spec
    return op


OP_D_ABS = _register_dve_ops()

F32 = mybir.dt.float32
BF16 = mybir.dt.bfloat16
Alu = mybir.AluOpType
Act = mybir.ActivationFunctionType

B, C, T = 4, 512, 4096
K, OK = 8, 3
O = 512  # C_out
H = 16  # left halo columns in the x slice
TH = 2048  # time columns per core
N_CORES = 8


def _strided(t, base_col, outer_step, outer_n, inner_n):
    """Overlapping AP over SBUF tile t: [128, outer_n, inner_n] where
    element [p, a, i] = t[p, base_col + a*outer_step + i]."""
    a = t[:, 0:inner_n].copy()
    pstep = tuple(list(a.ap)[0])
    a.ap = bass_rust.VecI64Pair(
        [pstep, (outer_step, outer_n), (1, inner_n)]
    )
    a.offset = base_col
    return a


def build_device_program(
    th=TH,
    tt=512,  # time chunk = one PSUM bank of fp32
    n_ct=4,  # contraction c-tiles of 128
    n_ot=4,  # output o-tiles of 128
):
    n_chunks = th // tt
    c_in = n_ct * 128
    o_out = n_ot * 128

    nc = bacc.Bacc("TRN2", target_bir_lowering=False, debug=False)

    x_d = nc.dram_tensor("xcore", [c_in, H + th], F32, kind="ExternalInput").ap()
    xr_d = nc.dram_tensor("xrot", [3, c_in, H + th], F32, kind="ExternalInput").ap()
    wt_d = nc.dram_tensor("wt", [n_ct, K, 128, o_out], BF16, kind="ExternalInput").ap()
    dgw_d = nc.dram_tensor(
        "diagw", [n_ct, K, OK, 128, 32], BF16, kind="ExternalInput"
    ).ap()
    offb_d = nc.dram_tensor("offb", [n_ct, 128, K], F32, kind="ExternalInput").ap()
    bias_d = nc.dram_tensor("biasr", [128, n_ot], F32, kind="ExternalInput").ap()
    out_d = nc.dram_tensor("out", [o_out, th], F32, kind="ExternalOutput").ap()

    W = H + tt  # working width incl halo
    QT = 4 * tt  # quad width

    units = []
    for chunk in range(n_chunks):
        for ct in range(n_ct):
            for q0 in (0, 1):
                units.append((chunk, ct, q0))

    with tile.TileContext(nc) as tc:
        with (
            tc.tile_pool(name="const", bufs=1) as cpool,
            tc.tile_pool(name="xb", bufs=3) as xbpool,
            tc.tile_pool(name="xrb", bufs=3) as xrpool,
            tc.tile_pool(name="chain", bufs=3) as chain,
            tc.tile_pool(name="spool", bufs=3) as spool,
            tc.tile_pool(name="outp", bufs=2) as outp,
            tc.tile_pool(name="psum", bufs=1, space="PSUM") as pspool,
            tc.tile_pool(name="rawps", bufs=4, space="PSUM") as rawps,
        ):
            # ---- resident constants ----
            wt_sb = []
            dgw_sb = []
            offb_sb = []
            for ct in range(n_ct):
                w = cpool.tile([128, K, o_out], BF16, tag=f"wt{ct}")
                nc.sync.dma_start(w[:], wt_d[ct].rearrange("k c o -> c k o"))
                wt_sb.append(w)
                g = cpool.tile([128, K, OK, 32], BF16, tag=f"dgw{ct}")
                nc.sync.dma_start(g[:], dgw_d[ct].rearrange("k j c o -> c k j o"))
                dgw_sb.append(g)
                ob = cpool.tile([128, K], F32, tag=f"offb{ct}")
                nc.sync.dma_start(ob[:], offb_d[ct])
                offb_sb.append(ob)
            bias_sb = cpool.tile([128, n_ot], F32, tag="biasr")
            nc.sync.dma_start(bias_sb[:], bias_d)

            xt = {}  # (chunk, ct) -> (xe[4], xo[4], De, Do)
            Svt = {}  # unit idx -> S tile
            ps = {}  # chunk -> {ot: psum tile}

            def emit_x_tiles(chunk, ct):
                xe, xo = [], []
                for r in range(4):
                    src = (
                        x_d[ct * 128 : (ct + 1) * 128]
                        if r == 0
                        else xr_d[r - 1, ct * 128 : (ct + 1) * 128]
                    )
                    pool = xbpool if r == 0 else xrpool
                    e = pool.tile([128, W], BF16, tag=f"Xe{r}")
                    nc.gpsimd.dma_start(e[:], src[:, chunk * tt : chunk * tt + W])
                    o = pool.tile([128, W], BF16, tag=f"Xo{r}")
                    nc.gpsimd.dma_start(
                        o[:, 0 : W - 1],
                        src[:, chunk * tt + 1 : chunk * tt + W],
                    )
                    xe.append(e)
                    xo.append(o)
                Xe, Xo = xe[0], xo[0]
                De = xbpool.tile([128, W], BF16, tag="De")
                nc.vector.tensor_tensor(
                    De[:, 2:W], Xe[:, 2:W], Xo[:, 0 : W - 2], Alu.subtract
                )
                Do = xbpool.tile([128, W], BF16, tag="Do")
                nc.vector.tensor_tensor(
                    Do[:, 0 : W - 2], Xo[:, 0 : W - 2], Xe[:, 0 : W - 2],
                    Alu.subtract,
                )
                xt[(chunk, ct)] = (xe, xo, De, Do)

            def emit_front(idx):
                """raw matmuls + d + p/S for unit idx."""
                chunk, ct, q0 = units[idx]
                if q0 == 0 and (chunk, ct) not in xt:
                    emit_x_tiles(chunk, ct)
                xe, xo, De, Do = xt[(chunk, ct)]

                def xs_rot(r, col, n=tt):
                    if col % 2 == 0:
                        return xe[r][:, col : col + n]
                    return xo[r][:, col - 1 : col - 1 + n]

                ks = [q0, q0 + 2, q0 + 4, q0 + 6]
                rp = {}
                for k in ks:
                    rp[k] = rawps.tile(
                        [128, tt], F32, tag="rawps", name=f"rp{idx}_{k}"
                    )
                for j in range(OK):
                    for i in range(4):
                        for k in ks:
                            r = k // 2
                            jc = (i + r) % 4
                            mov = xs_rot(r, H - 2 + j)
                            nc.tensor.matmul(
                                rp[k][32 * jc : 32 * jc + 32, :],
                                dgw_sb[ct][32 * i : 32 * i + 32, k, j, :],
                                mov[32 * i : 32 * i + 32, :],
                                start=(j == 0),
                                stop=(j == OK - 1),
                                tile_position=(32 * i, 32 * jc),
                                skip_group_check=True,
                            )
                # d = |raw + b|: ks[0], ks[2] on ScalarE; ks[1], ks[3] on VectorE
                dd = chain.tile([128, QT], BF16, tag="d")
                for qi, k in enumerate(ks):
                    dseg = dd[:, qi * tt : (qi + 1) * tt]
                    if qi % 2 == 0:
                        nc.scalar.activation(
                            dseg, rp[k][:], Act.Abs,
                            bias=offb_sb[ct][:, k : k + 1],
                        )
                    else:
                        nc.vector._custom_dve(
                            OP_D_ABS, out=dseg, in0=rp[k][:],
                            s0=offb_sb[ct][:, k : k + 1],
                        )

                # strided quad operands at cols k+9 (same parity)
                c0 = ks[0] + 9
                if c0 % 2 == 0:
                    pX = _strided(xe[0], c0, 2, 4, tt)
                    pD = _strided(De, c0, 2, 4, tt)
                else:
                    pX = _strided(xo[0], c0 - 1, 2, 4, tt)
                    pD = _strided(Do, c0 - 1, 2, 4, tt)

                def r4(t):
                    return t[:].rearrange("p (a b) -> p a b", a=4)

                p_t = chain.tile([128, QT], BF16, tag="p")
                nc.vector.tensor_tensor(r4(p_t), r4(dd), pD, Alu.mult)
                S_t = spool.tile([128, QT], BF16, tag="S")
                nc.vector.tensor_tensor(r4(S_t), pX, r4(p_t), Alu.subtract)
                Svt[idx] = S_t

            def emit_main(idx):
                chunk, ct, q0 = units[idx]
                if chunk not in ps:
                    ps[chunk] = {
                        ot: pspool.tile(
                            [128, tt], F32, tag=f"ps{ot}", name=f"ps{chunk}_{ot}"
                        )
                        for ot in range(n_ot)
                    }
                S_t = Svt.pop(idx)
                ks = [q0, q0 + 2, q0 + 4, q0 + 6]
                for qi, k in enumerate(ks):
                    first = ct == 0 and q0 == 0 and qi == 0
                    last = ct == n_ct - 1 and q0 == 1 and qi == 3
                    for ot in range(n_ot):
                        nc.tensor.matmul(
                            ps[chunk][ot][:],
                            wt_sb[ct][:, k, ot * 128 : (ot + 1) * 128],
                            S_t[:, qi * tt : (qi + 1) * tt],
                            start=first,
                            stop=last,
                        )
                if ct == n_ct - 1 and q0 == 1:  # last unit of chunk
                    pch = ps.pop(chunk)
                    for ot in range(n_ot):
                        out_sb = outp.tile([128, tt], F32, tag="osb")
                        nc.scalar.activation(
                            out_sb[:], pch[ot][:], Act.Identity,
                            bias=bias_sb[:, ot : ot + 1],
                        )
                        nc.sync.dma_start(
                            out_d[
                                ot * 128 : (ot + 1) * 128,
                                chunk * tt : (chunk + 1) * tt,
                            ],
                            out_sb[:],
                        )

            for idx in range(len(units) + 1):
                if idx < len(units):
                    emit_front(idx)
                if idx >= 1:
                    emit_main(idx - 1)

    nc.compile()
    return nc


def prep_host_inputs(x, offset_w, offset_b, weight, bias, th=TH):
    wt = (
        weight.transpose(1, 2, 0)  # [C, K, O]
        .reshape(4, 128, K, O)
        .transpose(0, 2, 1, 3)  # [ct, k, c, o]
        .astype(ml_dtypes.bfloat16)
    )
    wt = np.ascontiguousarray(wt)

    ow = offset_w.reshape(C, K, OK).astype(np.float32)  # [c, k, j]
    # diag 32-blocks with rotation-aware placement:
    # dgw[ct, k, j, p, f] = delta(p%32, f) * ow[ct*128 + 32*((p//32 + k//2)%4) + p%32, k, j]
    diagw = np.zeros((4, K, OK, 128, 32), ml_dtypes.bfloat16)
    q = np.arange(128) % 32
    blk = np.arange(128) // 32
    for ct in range(4):
        for k in range(K):
            ch = ct * 128 + 32 * ((blk + k // 2) % 4) + q
            for j in range(OK):
                diagw[ct, k, j, np.arange(128), q] = ow[ch, k, j].astype(
                    ml_dtypes.bfloat16
                )
    offb = np.ascontiguousarray(offset_b.reshape(4, 128, K).astype(np.float32))
    biasr = np.ascontiguousarray(bias.reshape(4, 128).T).astype(np.float32)

    xcores = []
    xrots = []
    n_th = T // th
    for core in range(N_CORES):
        b, thi = divmod(core, n_th)
        t0 = thi * th
        xc = np.zeros((C, H + th), np.float32)
        xc[:, H:] = x[b, :, t0 : t0 + th]
        if t0 >= H:
            xc[:, :H] = x[b, :, t0 - H : t0]
        xcores.append(np.ascontiguousarray(xc))
        # block-rotated copies: rot r partition p = channel (p + 32r) % 128
        xb4 = xc.reshape(4, 128, H + th)
        xr = np.empty((3, C, H + th), np.float32)
        for r in (1, 2, 3):
            idx = (np.arange(128) + 32 * r) % 128
            xr[r - 1] = xb4[:, idx, :].reshape(C, H + th)
        xrots.append(np.ascontiguousarray(xr))
    return wt, diagw, offb, biasr, xcores, xrots


_PROGRAM_CACHE = {}


def _get_program():
    key = "main"
    if key not in _PROGRAM_CACHE:
        _PROGRAM_CACHE[key] = build_device_program()
    return _PROGRAM_CACHE[key]


def run_on_hw(inputs, trace=False, **kw):
    from concourse.bass_utils import run_bass_kernel_spmd

    nc = _get_program()
    wt, diagw, offb, biasr, xcores, xrots = prep_host_inputs(
        inputs["x"], inputs["offset_w"], inputs["offset_b"],
        inputs["weight"], inputs["bias"],
    )
    in_maps = [
        {
            "xcore": xcores[core],
            "xrot": xrots[core],
            "wt": wt,
            "diagw": diagw,
            "offb": offb,
            "biasr": biasr,
        }
        for core in range(N_CORES)
    ]
    res = run_bass_kernel_spmd(
        nc, in_maps, core_ids=list(range(N_CORES)), trace=trace, **kw
    )
    return res


def kernel(**inputs) -> np.ndarray:
    res = run_on_hw(inputs)
    out = np.empty((B, O, T), np.float32)
    n_th = T // TH
    for core in range(N_CORES):
        b, thi = divmod(core, n_th)
        out[b, :, thi * TH : (thi + 1) * TH] = res.results[core]["out"]
    return out


if __name__ == "__main__":
    z = np.load("/root/problem/inputs.npz")
    out = kernel(**{k: z[k] for k in z.files})
    print("kernel out:", out.shape, out.dtype, float(np.abs(out).max()))


# revision 6
# speedup vs baseline: 1.9189x; 1.2767x over previous
"""Deformable causal conv1d Trainium2 kernel (v6).

Math (h-term dropped; 4.0e-3 rel): sampled[c,k,t] ~ a0 - d*D0 with
a0 = x[c,t+k-7], D[u] = x[u]-x[u-1], d = |raw+b|.

v6 = v5's 16-way tiled raw matmuls + software pipelining to keep the PE
dense (HAM stays at K=8/8):
  - Unit = (chunk, ct, q0-quad). PE stream: raw(u), main(u-1), raw(u+1),
    main(u), ...  main trails raw by one unit, so the d->p->S latency of
    unit u hides under raw(u+1) + main(u-1) instead of stalling the PE.
  - raw: 48 tile-MMs per unit on 16 concurrent 32x32 PE tiles
    (k of quad -> rotation r=k//2 -> col-group (i+r)%4, reading host-side
    block-rotated x copies; all 4 rotations present in every quad).
  - d = |raw+b|: 2 k's on ScalarE (Abs + bias), 2 k's on VectorE
    (tensor_scalar add-bias then abs_max 0) to halve the serial latency.
  - S-assembly: 2 bf16 TTs per quad (p = d*D0; S = a0 - p).
  - PSUM: 4 banks out accumulation + 4 banks raw (one unit in flight +
    one being drained) = 8.

Sharding: 8 cores = 4 batches x 2 time-halves. No collectives.
"""

import numpy as np
import ml_dtypes
import bass_rust

import concourse.bass as bass
import concourse.tile as tile
from concourse import bacc, mybir


def _register_dve_ops():
    import concourse.dve_ops as dops
    from concourse.dve_spec import Spec, Src0, C0, Zero, lower, maxx
    from concourse.dve_uop import DveOpSpec
    from concourse.dve_table_gen import dve_ver_for

    name = "DEFORM_D_ABS"
    for op in dops.OPS:
        if op.name == name:
            return op
    row = dops._CUSTOM_DVE_ROW_BASE + len(dops.OPS)
    assert row < 0x20
    dops._SUB_OPCODE_FOR_NAME[name] = row
    _t = Src0 + C0
    spec = Spec(
        body=maxx(_t, Zero - _t),
        reference=lambda in0, in1, s0, s1, imm2: np.abs(
            in0.astype(np.float32) + s0
        ),
    )
    shas = {}
    for ver in {dve_ver_for("TRN2"), dve_ver_for("TRN3")}:
        c = DveOpSpec(name=name, opcode=row, uops=lower(spec, ver=ver), rd1_en=False)
        shas[ver] = c.sha(ver)
    op = dops.DveOp(name, spec, subdim=False, uops_sha=shas)
    dops.OPS.append(op)
    dops.CUSTOM_DVE_SPECS[name] = spec
    return op


OP_D_ABS = _register_dve_ops()

F32 = mybir.dt.float32
BF16 = mybir.dt.bfloat16
Alu = mybir.AluOpType
Act = mybir.ActivationFunctionType

B, C, T = 4, 512, 4096
K, OK = 8, 3
O = 512  # C_out
H = 16  # left halo columns in the x slice
TH = 2048  # time columns per core
N_CORES = 8


def _strided(t, base_col, outer_step, outer_n, inner_n):
    """Overlapping AP over SBUF tile t: [128, outer_n, inner_n] where
    element [p, a, i] = t[p, base_col + a*outer_step + i]."""
    a = t[:, 0:inner_n].copy()
    pstep = tuple(list(a.ap)[0])
    a.ap = bass_rust.VecI64Pair(
        [pstep, (outer_step, outer_n), (1, inner_n)]
    )
    a.offset = base_col
    return a


def build_device_program(
    th=TH,
    tt=512,  # time chunk = one PSUM bank of fp32
    n_ct=4,  # contraction c-tiles of 128
    n_ot=4,  # output o-tiles of 128
):
    n_chunks = th // tt
    c_in = n_ct * 128
    o_out = n_ot * 128

    nc = bacc.Bacc("TRN2", target_bir_lowering=False, debug=False)

    x_d = nc.dram_tensor("xcore", [c_in, H + th], F32, kind="ExternalInput").ap()
    xr_d = nc.dram_tensor("xrot", [3, c_in, H + th], F32, kind="ExternalInput").ap()
    wt_d = nc.dram_tensor("wt", [n_ct, 128, K, o_out], BF16, kind="ExternalInput").ap()
    dgw_d = nc.dram_tensor(
        "diagw", [n_ct, 128, K, OK, 32], BF16, kind="ExternalInput"
    ).ap()
    offb_d = nc.dram_tensor("offb", [n_ct, 128, K], F32, kind="ExternalInput").ap()
    bias_d = nc.dram_tensor("biasr", [128, n_ot], F32, kind="ExternalInput").ap()
    out_d = nc.dram_tensor("out", [o_out, th], F32, kind="ExternalOutput").ap()

    W = H + tt  # working width incl halo
    QT = 4 * tt  # quad width

    units = []
    for chunk in range(n_chunks):
        for ct in range(n_ct):
            for q0 in (0, 1):
                units.append((chunk, ct, q0))

    with tile.TileContext(nc) as tc:
        with (
            tc.tile_pool(name="const", bufs=1) as cpool,
            tc.tile_pool(name="xb", bufs=3) as xbpool,
            tc.tile_pool(name="xrb", bufs=3) as xrpool,
            tc.tile_pool(name="chain", bufs=3) as chain,
            tc.tile_pool(name="spool", bufs=3) as spool,
            tc.tile_pool(name="outp", bufs=2) as outp,
            tc.tile_pool(name="psum", bufs=1, space="PSUM") as pspool,
            tc.tile_pool(name="rawps", bufs=4, space="PSUM") as rawps,
        ):
            # ---- resident constants ----
            wt_sb = []
            dgw_sb = []
            offb_sb = []
            # dgw/offb first so the first raw matmuls start ASAP; wt after.
            for ct in range(n_ct):
                g = cpool.tile([128, K, OK, 32], BF16, tag=f"dgw{ct}")
                nc.sync.dma_start(g[:], dgw_d[ct])
                dgw_sb.append(g)
                ob = cpool.tile([128, K], F32, tag=f"offb{ct}")
                nc.sync.dma_start(ob[:], offb_d[ct])
                offb_sb.append(ob)
            for ct in range(n_ct):
                w = cpool.tile([128, K, o_out], BF16, tag=f"wt{ct}")
                nc.sync.dma_start(w[:], wt_d[ct])
                wt_sb.append(w)
            bias_sb = cpool.tile([128, n_ot], F32, tag="biasr")
            nc.sync.dma_start(bias_sb[:], bias_d)

            xt = {}  # (chunk, ct) -> (xe[4], xo[4], De, Do)
            Svt = {}  # unit idx -> S tile
            ps = {}  # chunk -> {ot: psum tile}

            def emit_x_tiles(chunk, ct):
                xe, xo = [], []
                for r in range(4):
                    src = (
                        x_d[ct * 128 : (ct + 1) * 128]
                        if r == 0
                        else xr_d[r - 1, ct * 128 : (ct + 1) * 128]
                    )
                    pool = xbpool if r == 0 else xrpool
                    e = pool.tile([128, W], BF16, tag=f"Xe{r}")
                    nc.gpsimd.dma_start(e[:], src[:, chunk * tt : chunk * tt + W])
                    o = pool.tile([128, W], BF16, tag=f"Xo{r}")
                    nc.gpsimd.dma_start(
                        o[:, 0 : W - 1],
                        src[:, chunk * tt + 1 : chunk * tt + W],
                    )
                    xe.append(e)
                    xo.append(o)
                Xe, Xo = xe[0], xo[0]
                De = xbpool.tile([128, W], BF16, tag="De")
                nc.gpsimd.tensor_tensor(
                    De[:, 2:W], Xe[:, 2:W], Xo[:, 0 : W - 2], Alu.subtract
                )
                Do = xbpool.tile([128, W], BF16, tag="Do")
                nc.gpsimd.tensor_tensor(
                    Do[:, 0 : W - 2], Xo[:, 0 : W - 2], Xe[:, 0 : W - 2],
                    Alu.subtract,
                )
                xt[(chunk, ct)] = (xe, xo, De, Do)

            def emit_front(idx):
                """raw matmuls + d + p/S for unit idx."""
                chunk, ct, q0 = units[idx]
                if q0 == 0 and (chunk, ct) not in xt:
                    emit_x_tiles(chunk, ct)
                xe, xo, De, Do = xt[(chunk, ct)]

                def xs_rot(r, col, n=tt):
                    if col % 2 == 0:
                        return xe[r][:, col : col + n]
                    return xo[r][:, col - 1 : col - 1 + n]

                ks = [q0, q0 + 2, q0 + 4, q0 + 6]
                rp = {}
                for k in ks:
                    rp[k] = rawps.tile(
                        [128, tt], F32, tag="rawps", name=f"rp{idx}_{k}"
                    )
                for j in range(OK):
                    for i in range(4):
                        for k in ks:
                            r = k // 2
                            jc = (i + r) % 4
                            mov = xs_rot(r, H - 2 + j)
                            nc.tensor.matmul(
                                rp[k][32 * jc : 32 * jc + 32, :],
                                dgw_sb[ct][32 * i : 32 * i + 32, k, j, :],
                                mov[32 * i : 32 * i + 32, :],
                                start=(j == 0),
                                stop=(j == OK - 1),
                                tile_position=(32 * i, 32 * jc),
                                skip_group_check=True,
                            )
                # d = |raw + b|, then p/S per 2-k half so mains start early.
                # ks[0], ks[2] on ScalarE; ks[1], ks[3] on VectorE custom.
                dd = chain.tile([128, QT], BF16, tag="d")
                S_t = spool.tile([128, QT], BF16, tag="S")
                c0 = ks[0] + 9
                if c0 % 2 == 0:
                    tX, tD, cb = xe[0], De, c0
                else:
                    tX, tD, cb = xo[0], Do, c0 - 1

                def r2(t, half):
                    return t[:, half * 2 * tt : (half + 1) * 2 * tt].rearrange(
                        "p (a b) -> p a b", a=2
                    )

                for half in (0, 1):
                    for qi in (2 * half, 2 * half + 1):
                        k = ks[qi]
                        dseg = dd[:, qi * tt : (qi + 1) * tt]
                        if qi == 3:
                            nc.vector._custom_dve(
                                OP_D_ABS, out=dseg, in0=rp[k][:],
                                s0=offb_sb[ct][:, k : k + 1],
                            )
                        else:
                            nc.scalar.activation(
                                dseg, rp[k][:], Act.Abs,
                                bias=offb_sb[ct][:, k : k + 1],
                            )
                    pXh = _strided(tX, cb + 4 * half, 2, 2, tt)
                    pDh = _strided(tD, cb + 4 * half, 2, 2, tt)
                    p_t = chain.tile([128, 2 * tt], BF16, tag=f"p{half}")
                    nc.vector.tensor_tensor(r2(p_t, 0), r2(dd, half), pDh, Alu.mult)
                    nc.vector.tensor_tensor(r2(S_t, half), pXh, r2(p_t, 0), Alu.subtract)
                Svt[idx] = S_t

            def emit_main(idx):
                chunk, ct, q0 = units[idx]
                if chunk not in ps:
                    ps[chunk] = {
                        ot: pspool.tile(
                            [128, tt], F32, tag=f"ps{ot}", name=f"ps{chunk}_{ot}"
                        )
                        for ot in range(n_ot)
                    }
                S_t = Svt.pop(idx)
                ks = [q0, q0 + 2, q0 + 4, q0 + 6]
                for qi, k in enumerate(ks):
                    first = ct == 0 and q0 == 0 and qi == 0
                    last = ct == n_ct - 1 and q0 == 1 and qi == 3
                    for ot in range(n_ot):
                        nc.tensor.matmul(
                            ps[chunk][ot][:],
                            wt_sb[ct][:, k, ot * 128 : (ot + 1) * 128],
                            S_t[:, qi * tt : (qi + 1) * tt],
                            start=first,
                            stop=last,
                        )
                if ct == n_ct - 1 and q0 == 1:  # last unit of chunk
                    pch = ps.pop(chunk)
                    for ot in range(n_ot):
                        out_sb = outp.tile([128, tt], F32, tag="osb")
                        nc.scalar.activation(
                            out_sb[:], pch[ot][:], Act.Identity,
                            bias=bias_sb[:, ot : ot + 1],
                        )
                        eng = nc.sync if ot % 2 == 0 else nc.scalar
                        eng.dma_start(
                            out_d[
                                ot * 128 : (ot + 1) * 128,
                                chunk * tt : (chunk + 1) * tt,
                            ],
                            out_sb[:],
                        )

            for idx in range(len(units) + 1):
                if idx < len(units):
                    emit_front(idx)
                if idx >= 1:
                    emit_main(idx - 1)

    nc.compile()
    return nc


def prep_host_inputs(x, offset_w, offset_b, weight, bias, th=TH):
    wt = (
        weight.transpose(1, 2, 0)  # [C, K, O]
        .reshape(4, 128, K, O)  # [ct, c, k, o]
        .astype(ml_dtypes.bfloat16)
    )
    wt = np.ascontiguousarray(wt)

    ow = offset_w.reshape(C, K, OK).astype(np.float32)  # [c, k, j]
    # diag 32-blocks with rotation-aware placement:
    # dgw[ct, p, k, j, f] = delta(p%32, f) * ow[ct*128 + 32*((p//32 + k//2)%4) + p%32, k, j]
    diagw = np.zeros((4, 128, K, OK, 32), ml_dtypes.bfloat16)
    q = np.arange(128) % 32
    blk = np.arange(128) // 32
    for ct in range(4):
        for k in range(K):
            ch = ct * 128 + 32 * ((blk + k // 2) % 4) + q
            for j in range(OK):
                diagw[ct, np.arange(128), k, j, q] = ow[ch, k, j].astype(
                    ml_dtypes.bfloat16
                )
    offb = np.ascontiguousarray(offset_b.reshape(4, 128, K).astype(np.float32))
    biasr = np.ascontiguousarray(bias.reshape(4, 128).T).astype(np.float32)

    xcores = []
    xrots = []
    n_th = T // th
    for core in range(N_CORES):
        b, thi = divmod(core, n_th)
        t0 = thi * th
        xc = np.zeros((C, H + th), np.float32)
        xc[:, H:] = x[b, :, t0 : t0 + th]
        if t0 >= H:
            xc[:, :H] = x[b, :, t0 - H : t0]
        xcores.append(np.ascontiguousarray(xc))
        # block-rotated copies: rot r partition p = channel (p + 32r) % 128
        xb4 = xc.reshape(4, 128, H + th)
        xr = np.empty((3, C, H + th), np.float32)
        for r in (1, 2, 3):
            idx = (np.arange(128) + 32 * r) % 128
            xr[r - 1] = xb4[:, idx, :].reshape(C, H + th)
        xrots.append(np.ascontiguousarray(xr))
    return wt, diagw, offb, biasr, xcores, xrots


_PROGRAM_CACHE = {}


def _get_program():
    key = "main"
    if key not in _PROGRAM_CACHE:
        _PROGRAM_CACHE[key] = build_device_program()
    return _PROGRAM_CACHE[key]


def run_on_hw(inputs, trace=False, **kw):
    from concourse.bass_utils import run_bass_kernel_spmd

    nc = _get_program()
    wt, diagw, offb, biasr, xcores, xrots = prep_host_inputs(
        inputs["x"], inputs["offset_w"], inputs["offset_b"],
        inputs["weight"], inputs["bias"],
    )
    in_maps = [
        {
            "xcore": xcores[core],
            "xrot": xrots[core],
            "wt": wt,
            "diagw": diagw,
            "offb": offb,
            "biasr": biasr,
        }
        for core in range(N_CORES)
    ]
    res = run_bass_kernel_spmd(
        nc, in_maps, core_ids=list(range(N_CORES)), trace=trace, **kw
    )
    return res


def kernel(**inputs) -> np.ndarray:
    res = run_on_hw(inputs)
    out = np.empty((B, O, T), np.float32)
    n_th = T // TH
    for core in range(N_CORES):
        b, thi = divmod(core, n_th)
        out[b, :, thi * TH : (thi + 1) * TH] = res.results[core]["out"]
    return out


if __name__ == "__main__":
    z = np.load("/root/problem/inputs.npz")
    out = kernel(**{k: z[k] for k in z.files})
    print("kernel out:", out.shape, out.dtype, float(np.abs(out).max()))
